# revision 1
# baseline (speedup 1.0000x reference)
"""Multi-head graph attention (GAT-style) Trainium2 kernel, v2.

Problem: out[b,h,i,o] = softmax_j(mask(leakyrelu_0.2(src[b,h,i] + dst[b,h,j])))
         @ h_prime[b,h,:,:] + bias
with h_prime = h @ w[h], src/dst = tanh(h_prime) @ a_src/a_dst.

Key identity (exact): with x = src_i + dst_j,
    exp(leakyrelu_0.2(x)) = max(exp(x), exp(0.2x))
                          = exp(0.2 src_i) * max(g_i * D_j, d_j)
    g = exp(0.8 src), D = exp(dst), d = exp(0.2 dst)
The exp(0.2 src_i) prefactor is constant along the softmax axis j and
cancels, so the huge [N, H*N] leakyrelu+exp ACT passes of the naive
pipeline collapse to exp on a [24, 512] tile plus:
  - 8 tiny K=2 PE matmuls per graph: T'[j, h*128+i] = D_h[j] g_h[i] - d_h[j]
    (lhsT = [D;d] flat rows, rhs = [g-flat; -ones]); then
  - one fused DVE pass: Em = relu(T') * adjT   (scalar_tensor_tensor)
  - numerator = Em^T @ hpa + adjT^T @ (d ⊙ hpa)  accumulated in one PSUM
    (the second matmul restores the d_j floor of the max, pre-masked).
Row sums ride as a ones column in hpa; normalization happens on host
(output is the lambda-scaled numerator + sums, lambda = e^-6 keeps f16
in range; the scale cancels in the final division).

Data parallel over the 512-graph batch: 64 graphs per core, no
collectives.
"""

import numpy as np

BS, N, HEADS, DIN, DOUT = 512, 128, 8, 64, 64
NCORES = 8
BSH = BS // NCORES  # graphs per core
LOGLAM = -3.0  # ln(lambda) rescale so f16 numerators can't overflow
C1 = DOUT + 1  # 65: per-head out cols incl row-sum column

_cache = {}


def _build_nc():
    import concourse.bass as bass
    import concourse.mybir as mybir
    import concourse.tile as tile

    f32 = mybir.dt.float32
    f16 = mybir.dt.float16
    bf16 = mybir.dt.bfloat16
    AF = mybir.ActivationFunctionType
    ALU = mybir.AluOpType

    nc = bass.Bass("TRN2", target_bir_lowering=False, debug=False)

    # DRAM inputs (per core)
    hT_d = nc.dram_tensor("hT", [BSH, DIN, N], bf16, kind="ExternalInput").ap()
    adjT_d = nc.dram_tensor("adjT", [BSH, N, N], f16, kind="ExternalInput").ap()
    w_all_d = nc.dram_tensor("w_all", [DIN, HEADS * DOUT], bf16, kind="ExternalInput").ap()
    # a24: per pair-block p (heads 2p, 2p+1), cols 24p+r:
    #   r in [0,8): d-col for head r (rows q*64+o, q=r-2p), val 0.2*a_dst[r][o]
    #   r in [8,16): D-col, val a_dst
    #   r in [16,24): g-col, val 0.8*a_src
    a24_d = nc.dram_tensor("a24", [DIN * 2, 96], bf16, kind="ExternalInput").ap()
    ident8_d = nc.dram_tensor("ident8", [HEADS, HEADS], f16, kind="ExternalInput").ap()
    # DRAM output: out[b, i, h*65 + c]; c=64 is the softmax row sum
    out_d = nc.dram_tensor("out", [BSH, N, HEADS * C1], f16, kind="ExternalOutput").ap()
    import os
    DBG = bool(os.environ.get("KDEBUG"))
    if DBG:
        dbg_expS = nc.dram_tensor("dbg_expS", [24, 4 * N], f16, kind="ExternalOutput").ap()
        dbg_FL = nc.dram_tensor("dbg_FL", [2, HEADS * N], f16, kind="ExternalOutput").ap()
        dbg_rg = nc.dram_tensor("dbg_rg", [2, HEADS * N], f16, kind="ExternalOutput").ap()
        dbg_dT = nc.dram_tensor("dbg_dT", [N, HEADS], f16, kind="ExternalOutput").ap()
        dbg_Em = nc.dram_tensor("dbg_Em", [N, HEADS * N], f16, kind="ExternalOutput").ap()
        dbg_hpa = nc.dram_tensor("dbg_hpa", [N, HEADS * C1], f16, kind="ExternalOutput").ap()
        dbg_dhpa = nc.dram_tensor("dbg_dhpa", [N, HEADS * C1], f16, kind="ExternalOutput").ap()

    QUAD = 4  # graphs per S-phase block

    with tile.TileContext(nc) as tc:
        with (
            tc.tile_pool(name="consts", bufs=1) as cpool,
            tc.tile_pool(name="inbuf", bufs=2) as inpool,
            tc.tile_pool(name="adjbuf", bufs=3) as adjpool,
            tc.tile_pool(name="tts", bufs=2) as ttpool,
            tc.tile_pool(name="mid", bufs=2) as midpool,
            tc.tile_pool(name="em", bufs=2) as empool,
            tc.tile_pool(name="outbuf", bufs=3) as outpool,
            # PSUM banks (8): hpT 1 + S 1+1(dT) + T 2 + hp 1 + oa 2
            tc.tile_pool(name="ps_hpT", bufs=1, space="PSUM") as ps_hpT,
            tc.tile_pool(name="ps_S", bufs=1, space="PSUM") as ps_S,
            tc.tile_pool(name="ps_T", bufs=1, space="PSUM") as ps_T,
            tc.tile_pool(name="ps_hp", bufs=1, space="PSUM") as ps_hp,
            tc.tile_pool(name="ps_oa", bufs=2, space="PSUM") as ps_oa,
        ):
            # ---- constants (loaded once) ----
            w_all = cpool.tile([DIN, HEADS * DOUT], bf16, tag="w_all")
            nc.sync.dma_start(w_all[:], w_all_d[:])
            a24 = cpool.tile([DIN * 2, 96], bf16, tag="a24")
            nc.sync.dma_start(a24[:], a24_d[:])
            # per-partition bias for the exp over S24: rows 0-15 get ln(lam)
            bias24 = cpool.tile([24, 1], f32, tag="bias24")
            nc.gpsimd.memset(bias24[:], 0.0)
            nc.gpsimd.memset(bias24[0:16, :], LOGLAM)
            # rhs for the K=2 T' matmuls: row0 = g-flat (per graph), row1 = -1
            ident8 = cpool.tile([HEADS, HEADS], f16, tag="ident8")
            nc.sync.dma_start(ident8[:], ident8_d[:])
            rhsG = []
            for par in range(2):
                t = cpool.tile([2, HEADS * N], f16, tag=f"rhsG{par}")
                # row 1 is overwritten by the per-graph g-flat DMA; row 0
                # stays -1 (memset must start at partition 0, so cover both)
                nc.gpsimd.memset(t[:], -1.0)
                rhsG.append(t)

            nquads = BSH // QUAD
            for Q in range(nquads):
                b0 = Q * QUAD
                # hT for the quad: [i, b*128+n]
                hT_t = inpool.tile([DIN, QUAD * N], bf16, tag="hT")
                nc.sync.dma_start(
                    hT_t[:], hT_d[b0 : b0 + QUAD].rearrange("b i n -> i b n")
                )

                # ---- hpT + tanh per graph; tT packed [*, (p, b, n)] ----
                tT_t = ttpool.tile([128, 4 * QUAD * N], bf16, tag="tT")
                tT_v = tT_t[:].rearrange("P (p b n) -> P p b n", b=QUAD, n=N)
                for bq in range(QUAD):
                    hpT_ps = ps_hpT.tile([128, 4 * N], f32, tag="hpT")
                    for p in range(4):
                        nc.tensor.matmul(
                            hpT_ps[:, p * N : (p + 1) * N],
                            lhsT=w_all[:, p * 128 : (p + 1) * 128],
                            rhs=hT_t[:, bq * N : (bq + 1) * N],
                            start=True,
                            stop=True,
                        )
                    nc.scalar.activation(
                        tT_v[:, :, bq, :],
                        hpT_ps[:].rearrange("P (p n) -> P p n", n=N),
                        AF.Tanh,
                    )

                # ---- S24 for the quad: rows 0-7 d, 8-15 D, 16-23 g ----
                S_ps = ps_S.tile([24, QUAD * N], f32, tag="S24")
                for p in range(4):
                    nc.tensor.matmul(
                        S_ps[:],
                        lhsT=a24[:, 24 * p : 24 * (p + 1)],
                        rhs=tT_t[:, p * QUAD * N : (p + 1) * QUAD * N],
                        start=(p == 0),
                        stop=(p == 3),
                    )
                expS = midpool.tile([24, QUAD * N], f16, tag="expS")
                nc.scalar.activation(expS[:], S_ps[:], AF.Exp, bias=bias24[:])

                for bq in range(QUAD):
                    b = b0 + bq
                    qsl = slice(bq * N, (bq + 1) * N)

                    adjT_t = adjpool.tile([N, N], f16, tag="adjT")
                    nc.sync.dma_start(adjT_t[:], adjT_d[b])

                    # flatten [d;D] rows -> FL [2, 1024]; g row -> rhsG[parity]
                    # (row 0 of rhsG stays -1; row 1 is the g flat)
                    FL = midpool.tile([2, HEADS * N], f16, tag="FL")
                    nc.sync.dma_start(
                        FL[0:1, :].rearrange("r (h n) -> r h n", n=N),
                        expS[0:8, qsl],
                    )
                    nc.sync.dma_start(
                        FL[1:2, :].rearrange("r (h n) -> r h n", n=N),
                        expS[8:16, qsl],
                    )
                    rg = rhsG[bq % 2]
                    nc.sync.dma_start(
                        rg[1:2, :].rearrange("r (h n) -> r h n", n=N),
                        expS[16:24, qsl],
                    )
                    # dT[j, h] = d_h[j] via PE transpose (ident8) + DVE copy
                    dT_ps = ps_S.tile([N, HEADS], f16, tag="dTp")
                    nc.tensor.transpose(dT_ps[:], expS[0:8, qsl], ident8[:])
                    dT_sb = midpool.tile([N, HEADS], f16, tag="dT")
                    nc.vector.tensor_copy(dT_sb[:], dT_ps[:])

                    # ---- T'[j, h*128+i] = D_h[j] g_h[i] - d_h[j] (lhsT=[d;D]) ----
                    T_ps = ps_T.tile([N, HEADS * N], f32, tag="T")
                    for h in range(HEADS):
                        nc.tensor.matmul(
                            T_ps[:, h * N : (h + 1) * N],
                            lhsT=FL[:, h * N : (h + 1) * N],
                            rhs=rg[:, h * N : (h + 1) * N],
                            start=True,
                            stop=True,
                        )

                    # ---- Em = relu(T') * adjT, one fused DVE pass ----
                    Em = empool.tile([N, HEADS * N], f16, tag="Em")
                    nc.vector.scalar_tensor_tensor(
                        Em[:].rearrange("j (h i) -> j h i", i=N),
                        T_ps[:].rearrange("j (h i) -> j h i", i=N),
                        0.0,
                        adjT_t[:].unsqueeze(1).broadcast_to([N, HEADS, N]),
                        op0=ALU.max,
                        op1=ALU.mult,
                    )

                    # ---- h_prime natural + augmented (ones col per head) ----
                    hp_ps = ps_hp.tile([N, HEADS * DOUT], f32, tag="hp")
                    nc.tensor.matmul(
                        hp_ps[:], lhsT=hT_t[:, qsl], rhs=w_all[:],
                        start=True, stop=True,
                    )
                    hpa = midpool.tile([N, HEADS * C1], f16, tag="hpa")
                    hpa_v = hpa[:].rearrange("j (h c) -> j h c", c=C1)
                    nc.gpsimd.memset(hpa_v[:, :, DOUT], 1.0)
                    nc.scalar.activation(
                        hpa_v[:, :, 0:DOUT],
                        hp_ps[:].rearrange("j (h c) -> j h c", c=DOUT),
                        AF.Copy,
                    )
                    # dhpa = d ⊙ hpa (for the d_j floor term), on GPSIMD
                    dhpa = midpool.tile([N, HEADS * C1], f16, tag="dhpa")
                    nc.gpsimd.tensor_tensor(
                        dhpa[:].rearrange("j (h c) -> j h c", c=C1),
                        hpa_v[:],
                        dT_sb[:].unsqueeze(2).broadcast_to([N, HEADS, C1]),
                        ALU.mult,
                    )

                    if DBG and Q == 0 and bq == 0:
                        nc.sync.dma_start(dbg_expS[:], expS[:])
                        nc.sync.dma_start(dbg_FL[:], FL[:])
                        nc.sync.dma_start(dbg_rg[:], rg[:])
                        nc.sync.dma_start(dbg_dT[:], dT_sb[:])
                        nc.sync.dma_start(dbg_Em[:], Em[:])
                        nc.sync.dma_start(dbg_hpa[:], hpa[:])
                        nc.sync.dma_start(dbg_dhpa[:], dhpa[:])

                    # ---- numerator + row sums ----
                    oa_lo = ps_oa.tile([N, 4 * C1], f32, tag="oa")
                    oa_hi = ps_oa.tile([N, 4 * C1], f32, tag="oa")
                    # d-floor term first (start=True over the full half),
                    # then each head's Em term accumulates and stops its region
                    nc.tensor.matmul(
                        oa_lo[:], lhsT=adjT_t[:], rhs=dhpa[:, 0 : 4 * C1],
                        start=True, stop=False,
                    )
                    nc.tensor.matmul(
                        oa_hi[:], lhsT=adjT_t[:], rhs=dhpa[:, 4 * C1 : 8 * C1],
                        start=True, stop=False,
                    )
                    for h in range(HEADS):
                        oa = oa_lo if h < 4 else oa_hi
                        c0 = (h % 4) * C1
                        nc.tensor.matmul(
                            oa[:, c0 : c0 + C1],
                            lhsT=Em[:, h * N : (h + 1) * N],
                            rhs=hpa[:, h * C1 : (h + 1) * C1],
                            start=False,
                            stop=True,
                        )

                    # ---- PSUM -> SBUF (split ACT/DVE), DMA out ----
                    out_sb = outpool.tile([N, HEADS * C1], f16, tag="out_sb")
                    nc.scalar.activation(out_sb[:, 0 : 4 * C1], oa_lo[:], AF.Copy)
                    nc.vector.tensor_copy(out_sb[:, 4 * C1 : 8 * C1], oa_hi[:])
                    nc.sync.dma_start(out_d[b], out_sb[:])

    _split_excess_waits(nc)
    return nc


def _split_excess_waits(nc, cap=1):
    """Walrus codegen accepts at most `cap` sync-wait commands per
    instruction; hoist excess waits onto standalone drains inserted before."""
    import concourse.mybir as mybir

    n_new = 0
    for _bbname, bbw in nc.bb_map.items():
        inner = bbw.bb
        il = list(inner.instructions)
        out, changed = [], False
        for inst in il:
            si = inst.sync_info
            waits = list(si.on_wait) if si and si.on_wait else []
            if len(waits) > cap:
                extra = waits[:-cap]
                for ci in range(0, len(extra), cap):
                    chunk = extra[ci : ci + cap]
                    nop = mybir.InstDrain(
                        name=f"{inst.name}_wsplit{ci}", ins=[], outs=[],
                        bass_is_fusable=False,
                    )
                    nop.engine = inst.engine
                    nop.sync_info = mybir.SyncInfo(on_wait=chunk, on_update=[])
                    nc.register_instruction(nop)
                    out.append(nop)
                    n_new += 1
                si.on_wait = waits[-cap:]
                changed = True
            out.append(inst)
        if changed:
            inner.instructions = out
    return n_new


def _host_prep(h, adj, w, a_src, a_dst):
    import ml_dtypes

    bf = ml_dtypes.bfloat16
    hT = np.ascontiguousarray(h.transpose(0, 2, 1)).astype(bf)  # [BS, DIN, N]
    adjT = np.ascontiguousarray(
        adj.transpose(0, 2, 1).astype(np.float16)
    )  # [BS, j, i] as 0/1 f16
    w_all = np.ascontiguousarray(w.transpose(1, 0, 2).reshape(DIN, HEADS * DOUT)).astype(bf)
    a24 = np.zeros((DIN * 2, 96), np.float32)
    for p in range(4):
        for q in range(2):
            hh = 2 * p + q
            rows = slice(q * DIN, (q + 1) * DIN)
            a24[rows, 24 * p + hh] = 0.2 * a_dst[hh, :, 0]
            a24[rows, 24 * p + 8 + hh] = a_dst[hh, :, 0]
            a24[rows, 24 * p + 16 + hh] = 0.8 * a_src[hh, :, 0]
    a24 = a24.astype(bf)
    return hT, adjT, w_all, a24


def _make_in_maps(h, adj, w, a_src, a_dst):
    hT, adjT, w_all, a24 = _host_prep(h, adj, w, a_src, a_dst)
    in_maps = []
    for c in range(NCORES):
        sl = slice(c * BSH, (c + 1) * BSH)
        in_maps.append(
            {
                "hT": np.ascontiguousarray(hT[sl]),
                "adjT": np.ascontiguousarray(adjT[sl]),
                "w_all": w_all,
                "a24": a24,
                "ident8": np.eye(HEADS, dtype=np.float16),
            }
        )
    return in_maps


def _gather(results, bias):
    # results[c]["out"]: [BSH, N, HEADS*65]; c=64 holds the row sum
    full = np.concatenate([results[c]["out"] for c in range(NCORES)], axis=0)
    full = full.reshape(BS, N, HEADS, C1).astype(np.float32)
    out = full[..., 0:DOUT] / full[..., DOUT:DOUT + 1]
    out = out.transpose(0, 2, 1, 3)
    return np.ascontiguousarray(out + bias[None, None, None, :]).astype(np.float32)


def kernel(h, adj, w, a_src, a_dst, bias, _trace=False):
    from concourse.bass_utils import run_bass_kernel_spmd

    h = np.asarray(h, np.float32)
    adj = np.asarray(adj, bool)
    w = np.asarray(w, np.float32)
    a_src = np.asarray(a_src, np.float32)
    a_dst = np.asarray(a_dst, np.float32)
    bias = np.asarray(bias, np.float32)

    if "nc" not in _cache:
        _cache["nc"] = _build_nc()
    nc = _cache["nc"]

    in_maps = _make_in_maps(h, adj, w, a_src, a_dst)
    res = run_bass_kernel_spmd(nc, in_maps, core_ids=list(range(NCORES)), trace=_trace)
    out = _gather(res.results, bias)
    if _trace:
        _cache["last_result"] = res
    return out

